# revision 17
# baseline (speedup 1.0000x reference)
"""Trainium2 Bass kernel for nn_CNN_symmetry (dense_cnn).

Strategy:
  * Pure data parallelism: batch B=32768 sharded across 8 NeuronCores (4096 each).
  * Host-side (numpy): build the 144x144 banded conv matrices from the tiny
    13x4 weights, pre-transpose them into lhsT blocks; rearrange dots into
    [pixel, batch] bf16 planes (main 128 pixels + tail 16 pixels).
  * Device: convs as TensorE matmuls (bf16 in, fp32 PSUM). Convolution pairs
    that share an rhs (cne+cepl on e; cne+csum on m_c) get their 16-row tail
    outputs stacked into one [32,N] psum, saving 2 of 8 matmul passes per pair.
    Elementwise is spread across DVE / ACT / GPSIMD; chunks are stage-skewed
    and double-buffered so engine queues always hold another chunk's work.

Algebraic restructuring (validated vs reference in fp64):
    e=[x==0], m_c=[x==c];  C_sum=C_each+C_ne, C_epl=C_emp+C_ne, T=C_ne@1
    ne_e=C_ne@e ; t0=T-ne_e ; Ew'=C_epl@e-T (=E-t0) ; ie=1-e
    NECn_c = C_ne@m_c - t0        (= -NEC_c)
    s0 = sum_c m_c*(C_sum@m_c + Ew')   [select: masks disjoint]
    all_v = sigmoid(s0);  Ebar = ie*(Ew'+t0)
    2x: g_c=-NECn_c*all_v; s += Ebar + sum_c m_c*(C_ne2@g_c); all_v=tanh(s/2)
    out = lrelu(lrelu(all_v@W1')@W2'+b2)@W3'+b3
"""

import os
import sys
from contextlib import ExitStack

import numpy as np

sys.path.insert(0, "/opt/trn_rl_repo")
os.environ.setdefault("MYCRO_LOCAL_CACHE", "1")

import ml_dtypes  # noqa: E402

import concourse.bass as bass  # noqa: E402
import concourse.bacc as bacc  # noqa: E402
import concourse.tile as tile  # noqa: E402
from concourse import mybir  # noqa: E402

V, H, B = 14, 12, 32768
NK, CT = 5, 4
HALF, FULL = 6, 13
NPIX, NSQ = 144, 100
NCORES = 8
BC = B // NCORES          # 4096 per core
CHUNK = 1024              # batch chunk processed per pipeline pass
NCHUNK = BC // CHUNK
PM, PT = 128, 16          # main/tail pixel split (i-major order p = i*12 + j)

BF16 = mybir.dt.bfloat16
F32 = mybir.dt.float32
I16 = mybir.dt.int16
AF = mybir.ActivationFunctionType
ALU = mybir.AluOpType

# select routing per color (c>=1): A = ACT copy + DVE mult/add, D = DVE pred
S0_ROUTE = ("-", "D", "D", "D", "D")
ACC_ROUTE = ("-", "D", "D", "D", "D")
G_GPS = (False, False, False, False, False)  # GPSIMD elementwise contends with DVE SBUF port: keep off


# ---------------------------------------------------------------- host prep

def _build_K(W):
    Wa = np.abs(np.asarray(W, np.float64))
    K = np.zeros((FULL, FULL))
    K[:, HALF:HALF + CT] = Wa
    K[:, HALF - CT + 1:HALF + 1] = Wa[:, ::-1]
    return K


def _band(K):
    C = np.zeros((NPIX, NPIX))
    for i in range(12):
        for j in range(12):
            for i2 in range(12):
                for j2 in range(12):
                    di, dj = i2 - i + HALF, j2 - j + HALF
                    if 0 <= di < FULL and 0 <= dj < FULL:
                        C[i * 12 + j, i2 * 12 + j2] = K[di, dj]
    return C


def _lhsT(C):
    """C (out,in) -> lhsT = C.T blocks: mm [128,128], tm [16,128], mt [128,16], tt [16,16]."""
    L = C.T.astype(np.float64)
    return {
        "mm": L[:PM, :PM], "tm": L[PM:, :PM],
        "mt": L[:PM, PM:], "tt": L[PM:, PM:],
    }


def build_consts(w_each, w_not_each, w_not_each_2nd, w_empty, W1, W2, b2, W3, b3):
    C_each = _band(_build_K(w_each))
    C_ne = _band(_build_K(w_not_each))
    C_ne2 = _band(_build_K(w_not_each_2nd))
    C_emp = _band(_build_K(w_empty))
    C_sum = C_each + C_ne
    C_epl = C_emp + C_ne
    T = C_ne @ np.ones(NPIX)

    bf = lambda a: np.ascontiguousarray(np.asarray(a), dtype=ml_dtypes.bfloat16)
    f32 = lambda a: np.ascontiguousarray(np.asarray(a), dtype=np.float32)

    Bne, Bsum, Bepl, Bne2 = _lhsT(C_ne), _lhsT(C_sum), _lhsT(C_epl), _lhsT(C_ne2)
    consts = {}
    for nm, Bk in (("cne", Bne), ("csum", Bsum), ("cepl", Bepl), ("cne2", Bne2)):
        consts[f"{nm}_mm"] = bf(Bk["mm"])
        consts[f"{nm}_tm"] = bf(Bk["tm"])
    consts["cne2_mt"] = bf(Bne2["mt"])
    consts["cne2_tt"] = bf(Bne2["tt"])

    def mrg(Ab, Bb, k):
        out = np.zeros((Ab[k].shape[0], 48))
        out[:, :PT] = Ab[k]
        out[:, 32:48] = Bb[k]
        return bf(out)

    consts["necp_mt"] = mrg(Bne, Bepl, "mt")
    consts["necp_tt"] = mrg(Bne, Bepl, "tt")
    consts["necs_mt"] = mrg(Bne, Bsum, "mt")
    consts["necs_tt"] = mrg(Bne, Bsum, "tt")
    consts["t_m"] = f32(T[:PM].reshape(PM, 1))
    consts["t_t"] = f32(T[PM:].reshape(PT, 1))
    consts["tn_m"] = f32(-T[:PM].reshape(PM, 1))
    consts["tn_t"] = f32(-T[PM:].reshape(PT, 1))
    W1T = np.asarray(W1, np.float64).T        # [144, 100]
    consts["w1_m"] = bf(W1T[:PM])
    consts["w1_t"] = bf(W1T[PM:])
    consts["w2"] = bf(np.asarray(W2, np.float64).T)   # [100, 100]
    consts["w3"] = bf(np.asarray(W3, np.float64).T)   # [100, 1]
    consts["b2"] = f32(np.asarray(b2).reshape(NSQ, 1))
    consts["b3"] = f32(np.asarray(b3).reshape(1, 1))
    return consts


CONST_SPECS = (
    [(f"{n}_mm", [PM, PM], BF16) for n in ("cne", "csum", "cepl", "cne2")]
    + [(f"{n}_tm", [PT, PM], BF16) for n in ("cne", "csum", "cepl", "cne2")]
    + [("cne2_mt", [PM, PT], BF16), ("cne2_tt", [PT, PT], BF16),
       ("necp_mt", [PM, 48], BF16), ("necp_tt", [PT, 48], BF16),
       ("necs_mt", [PM, 48], BF16), ("necs_tt", [PT, 48], BF16),
    
       ("t_m", [PM, 1], F32), ("t_t", [PT, 1], F32),
       ("tn_m", [PM, 1], F32), ("tn_t", [PT, 1], F32),
       ("w1_m", [PM, NSQ], BF16), ("w1_t", [PT, NSQ], BF16),
       ("w2", [NSQ, NSQ], BF16), ("w3", [NSQ, 1], BF16),
       ("b2", [NSQ, 1], F32), ("b3", [1, 1], F32)]
)


# ---------------------------------------------------------------- device kernel

def emit_kernel(nc, bc, chunk):
    nchunk = bc // chunk
    xm_d = nc.dram_tensor("xm", [PM, bc], BF16, kind="ExternalInput")
    xt_d = nc.dram_tensor("xt", [PT, bc], BF16, kind="ExternalInput")
    out_d = nc.dram_tensor("out", [1, bc], F32, kind="ExternalOutput")
    const_d = {n: nc.dram_tensor(n, shp, dt, kind="ExternalInput")
               for n, shp, dt in CONST_SPECS}

    with tile.TileContext(nc) as tc, ExitStack() as ctx:
        cpool = ctx.enter_context(tc.tile_pool(name="consts", bufs=1))
        xpool = ctx.enter_context(tc.tile_pool(name="x", bufs=1))
        mpool = ctx.enter_context(tc.tile_pool(name="masks", bufs=2))
        npool = ctx.enter_context(tc.tile_pool(name="necn", bufs=2))
        spool = ctx.enter_context(tc.tile_pool(name="smisc", bufs=2))
        zpool = ctx.enter_context(tc.tile_pool(name="zscr", bufs=2))
        gpool = ctx.enter_context(tc.tile_pool(name="g", bufs=3))
        ppool = ctx.enter_context(tc.tile_pool(name="ps", bufs=2, space="PSUM"))
        tpool = ctx.enter_context(tc.tile_pool(name="pst", bufs=2, space="PSUM"))
        opool = ctx.enter_context(tc.tile_pool(name="outs", bufs=2))
        ypool = ctx.enter_context(tc.tile_pool(name="yout", bufs=1))

        # ACT table-set warmup (sigmoid/tanh share a set; load it once early).
        warm = cpool.tile([1, 1], F32, tag="warm", name="warm")
        nc.vector.memset(warm[:], 0.0)
        nc.scalar.activation(warm[:], warm[:], AF.Copy)
        nc.scalar.activation(warm[:], warm[:], AF.Sigmoid)
        nc.scalar.activation(warm[:], warm[:], AF.Tanh)

        # first const (feeds PE warmup) + inputs on the sync HWDGE ring, so
        # chunk 0's masks can start ASAP; remaining consts via gpsimd SWDGE.
        C = {}
        n0, shp0, dt0 = CONST_SPECS[0]
        t = cpool.tile(shp0, dt0, tag=n0, name=n0)
        nc.sync.dma_start(t[:], const_d[n0][:])
        C[n0] = t

        xm_f = xpool.tile([PM, bc], BF16, tag="xm", name="xm_t")
        xt_f = xpool.tile([PT, bc], BF16, tag="xt", name="xt_t")
        for ck in range(nchunk):
            sl = slice(ck * chunk, (ck + 1) * chunk)
            nc.sync.dma_start(xm_f[:, sl], xm_d[:, sl])
            nc.sync.dma_start(xt_f[:, sl], xt_d[:, sl])

        for n, shp, dt in CONST_SPECS[1:]:
            t = cpool.tile(shp, dt, tag=n, name=n)
            nc.gpsimd.dma_start(t[:], const_d[n][:])
            C[n] = t

        # PE warmup while input DMAs land (HAM un-throttle needs ~3.4us busy).
        wps = ppool.tile([PM, chunk], F32, tag="psm", name="warm_ps")
        for _ in range(48):
            nc.tensor.matmul(wps[:, :128], C[n0][:], C[n0][:],
                             start=True, stop=True)

        def chunk_stages(ck):
            c0 = ck * chunk
            xm = xm_f[:, c0:c0 + chunk]
            xt = xt_f[:, c0:c0 + chunk]

            # --- S0: masks (bf16 0/1) + ie ---------------------------------
            masks = []  # (main, tail) pairs; masks[0] is e
            for c in range(NK + 1):
                mm_ = mpool.tile([PM, chunk], BF16, tag=f"mkm{c}", name=f"mkm{c}")
                mt_ = mpool.tile([PT, chunk], BF16, tag=f"mkt{c}", name=f"mkt{c}")
                nc.vector.tensor_scalar(mm_[:], xm[:], float(c), None, ALU.is_equal)
                nc.vector.tensor_scalar(mt_[:], xt[:], float(c), None, ALU.is_equal)
                masks.append((mm_, mt_))
            ie = (spool.tile([PM, chunk], BF16, tag="iem", name="iem"),
                  spool.tile([PT, chunk], BF16, tag="iet", name="iet"))
            nc.vector.tensor_scalar(ie[0][:], xm[:], 0.0, None, ALU.not_equal)
            nc.vector.tensor_scalar(ie[1][:], xt[:], 0.0, None, ALU.not_equal)
            yield

            def tiles(tag, pool=spool):
                return (pool.tile([PM, chunk], BF16, tag=tag + "m", name=tag + "m"),
                        pool.tile([PT, chunk], BF16, tag=tag + "t", name=tag + "t"))

            def mm_main(mat, rhs_m, rhs_t):
                """main-output conv part: K=144 split as mm(128)+tm(16)."""
                ps = ppool.tile([PM, chunk], F32, tag="psm", name="psm")
                for nn in range(0, chunk, 512):
                    nc.tensor.matmul(ps[:, nn:nn + 512], C[f"{mat}_mm"][:],
                                     rhs_m[:, nn:nn + 512], start=True, stop=False)
                for nn in range(0, chunk, 512):
                    nc.tensor.matmul(ps[:, nn:nn + 512], C[f"{mat}_tm"][:],
                                     rhs_t[:, nn:nn + 512], start=False, stop=True)
                return ps

            def mm_tail(mat, width, rhs_m, rhs_t):
                """tail-output conv part (possibly merged pair): [width,N]."""
                ps = tpool.tile([48, chunk], F32, tag="pst", name="pst")
                for nn in range(0, chunk, 512):
                    nc.tensor.matmul(ps[:width, nn:nn + 512], C[f"{mat}_mt"][:],
                                     rhs_m[:, nn:nn + 512], start=True, stop=False)
                for nn in range(0, chunk, 512):
                    nc.tensor.matmul(ps[:width, nn:nn + 512], C[f"{mat}_tt"][:],
                                     rhs_t[:, nn:nn + 512], start=False, stop=True)
                return ps

            def zscr(p):
                return (zpool.tile([PM, chunk], BF16, tag="zm", name="zm")
                        if p == 0 else
                        zpool.tile([PT, chunk], BF16, tag="zt", name="zt"))

            # --- S1: e-convs -> t0, Ew', iew, ebar -------------------------
            t0 = tiles("t0")
            ew = tiles("ew")
            ps = mm_main("cne", masks[0][0], masks[0][1])
            nc.scalar.activation(t0[0][:], ps[:], AF.Identity,
                                 bias=C["t_m"][:], scale=-1.0)
            ps = mm_main("cepl", masks[0][0], masks[0][1])
            nc.scalar.activation(ew[0][:], ps[:], AF.Identity, bias=C["tn_m"][:])
            ps = mm_tail("necp", 48, masks[0][0], masks[0][1])
            nc.scalar.activation(t0[1][:], ps[:PT], AF.Identity,
                                 bias=C["t_t"][:], scale=-1.0)
            nc.scalar.activation(ew[1][:], ps[32:48], AF.Identity,
                                 bias=C["tn_t"][:])
            iew = tiles("iew")
            ebar = tiles("ebar")
            for r in range(2):
                nc.vector.tensor_tensor(iew[r][:], ie[r][:], ew[r][:], ALU.mult)
                nc.vector.tensor_tensor(ebar[r][:], ew[r][:], t0[r][:], ALU.add)
                nc.vector.tensor_tensor(ebar[r][:], ebar[r][:], ie[r][:], ALU.mult)
            yield

            # --- S2+S3: per color, merged cne/csum convs:
            #     necn_c = C_ne@m_c - t0 ; s0 = select_c(C_sum@m_c) + iew ----
            necn = [tiles(f"necn{c}", pool=npool) for c in range(NK)]
            s = tiles("s")

            def sel_step(route, dst, mk, src_ap, first, p):
                """dst := select(mk, src) accumulate; masks disjoint."""
                if first:
                    nc.vector.tensor_tensor(dst[:], src_ap, mk[:], ALU.mult)
                elif route == "A":
                    z = zscr(p)
                    nc.scalar.activation(z[:], src_ap, AF.Copy)
                    zz = zscr(p)
                    nc.vector.tensor_tensor(zz[:], z[:], mk[:], ALU.mult)
                    nc.vector.tensor_tensor(dst[:], dst[:], zz[:], ALU.add)
                else:
                    nc.vector.copy_predicated(dst[:], mk[:].bitcast(I16), src_ap)

            def color_pair(c):
                ps = mm_main("cne", masks[c + 1][0], masks[c + 1][1])
                if c >= 3:
                    nc.vector.tensor_tensor(necn[c][0][:], ps[:], t0[0][:],
                                            ALU.subtract)
                else:
                    z = zscr(0)
                    nc.scalar.activation(z[:], ps[:], AF.Copy)
                    nc.vector.tensor_tensor(necn[c][0][:], z[:], t0[0][:],
                                            ALU.subtract)
                ps = mm_main("csum", masks[c + 1][0], masks[c + 1][1])
                sel_step(S0_ROUTE[c], s[0], masks[c + 1][0], ps[:], c == 0, 0)
                ps = mm_tail("necs", 48, masks[c + 1][0], masks[c + 1][1])
                z = zscr(1)
                nc.scalar.activation(z[:], ps[:PT], AF.Copy)
                nc.vector.tensor_tensor(necn[c][1][:], z[:], t0[1][:], ALU.subtract)
                sel_step(S0_ROUTE[c], s[1], masks[c + 1][1], ps[32:48],
                         c == 0, 1)

            for c in range(3):
                color_pair(c)
            yield  # end S2

            for c in range(3, NK):
                color_pair(c)
            allv = tiles("allv")
            for r in range(2):
                nc.vector.tensor_tensor(s[r][:], s[r][:], iew[r][:], ALU.add)
                nc.scalar.activation(allv[r][:], s[r][:], AF.Sigmoid)
            yield  # end S3

            # --- S4/S5: depth iterations -----------------------------------
            for it in range(2):
                acc = tiles("acc")
                for c in range(NK):
                    g = (gpool.tile([PM, chunk], BF16, tag="gm", name="gm"),
                         gpool.tile([PT, chunk], BF16, tag="gt", name="gt"))
                    eng_m = nc.gpsimd if G_GPS[c] else nc.vector
                    eng_m.tensor_tensor(g[0][:], necn[c][0][:], allv[0][:],
                                        ALU.mult)
                    nc.vector.tensor_tensor(g[1][:], necn[c][1][:], allv[1][:],
                                            ALU.mult)
                    ps = mm_main("cne2", g[0], g[1])
                    sel_step(ACC_ROUTE[c], acc[0], masks[c + 1][0], ps[:],
                             c == 0, 0)
                    ps = mm_tail("cne2", PT, g[0], g[1])
                    sel_step(ACC_ROUTE[c], acc[1], masks[c + 1][1], ps[:PT],
                             c == 0, 1)
                for r in range(2):
                    nc.vector.tensor_tensor(s[r][:], s[r][:], ebar[r][:], ALU.add)
                    nc.vector.tensor_tensor(s[r][:], s[r][:], acc[r][:],
                                            ALU.subtract)
                    nc.scalar.activation(allv[r][:], s[r][:], AF.Tanh, scale=0.5)
                yield  # end S4 / S5

            # --- S6: MLP ----------------------------------------------------
            h1 = opool.tile([NSQ, chunk], BF16, tag="h1", name="h1")
            ps = ppool.tile([PM, chunk], F32, tag="psm", name="psm")
            for nn in range(0, chunk, 512):
                sl = slice(nn, nn + 512)
                nc.tensor.matmul(ps[:NSQ, nn:nn + 512], C["w1_m"][:],
                                 allv[0][:, sl], start=True, stop=False)
                nc.tensor.matmul(ps[:NSQ, nn:nn + 512], C["w1_t"][:],
                                 allv[1][:, sl], start=False, stop=True)
            nc.scalar.activation(h1[:], ps[:NSQ], AF.Copy)
            nc.vector.scalar_tensor_tensor(h1[:], ps[:NSQ], 0.2, h1[:],
                                           ALU.mult, ALU.max)

            h2 = opool.tile([NSQ, chunk], BF16, tag="h2", name="h2")
            ps = ppool.tile([PM, chunk], F32, tag="psm", name="psm")
            for nn in range(0, chunk, 512):
                nc.tensor.matmul(ps[:NSQ, nn:nn + 512], C["w2"][:],
                                 h1[:, nn:nn + 512], start=True, stop=True)
            nc.scalar.activation(h2[:], ps[:NSQ], AF.Identity, bias=C["b2"][:])
            nc.vector.scalar_tensor_tensor(h2[:], h2[:], 0.2, h2[:],
                                           ALU.mult, ALU.max)

            yout = ypool.tile([1, chunk], F32, tag="yout", name="yout")
            ps = tpool.tile([48, chunk], F32, tag="pst", name="pst")
            for nn in range(0, chunk, 512):
                nc.tensor.matmul(ps[:1, nn:nn + 512], C["w3"][:],
                                 h2[:, nn:nn + 512], start=True, stop=True)
            nc.scalar.activation(yout[:], ps[:1], AF.Identity, bias=C["b3"][:])
            nc.sync.dma_start(out_d[:, c0:c0 + chunk], yout[:])
            yield  # end S6

        # software pipeline: skewed stage interleave across chunks.
        NSTAGE, SKEW = 7, 3
        gens = [chunk_stages(ck) for ck in range(nchunk)]
        sched = sorted((ck * SKEW + st, ck, st)
                       for ck in range(nchunk) for st in range(NSTAGE))
        for _, ck, _s in sched:
            next(gens[ck], None)

    return nc


# ---------------------------------------------------------------- entry point

def _prep_inputs(dots):
    """dots (14,12,B) int32 -> per-core bf16 xm [128, BC], xt [16, BC]."""
    x = np.asarray(dots)[:12].reshape(NPIX, B).astype(ml_dtypes.bfloat16)
    xms, xts = [], []
    for k in range(NCORES):
        sl = x[:, k * BC:(k + 1) * BC]
        xms.append(np.ascontiguousarray(sl[:PM]))
        xts.append(np.ascontiguousarray(sl[PM:]))
    return xms, xts


def kernel(dots, w_each, w_not_each, w_not_each_2nd, w_empty, W1, W2, b2, W3, b3):
    from concourse.bass_utils import run_bass_kernel_spmd

    consts = build_consts(w_each, w_not_each, w_not_each_2nd, w_empty,
                          W1, W2, b2, W3, b3)
    xms, xts = _prep_inputs(dots)

    nc = bacc.Bacc()
    emit_kernel(nc, BC, CHUNK)
    nc.compile()

    in_maps = [dict(consts, xm=xms[k], xt=xts[k]) for k in range(NCORES)]
    res = run_bass_kernel_spmd(nc, in_maps, list(range(NCORES)))
    out = np.concatenate([np.asarray(r["out"]).reshape(BC) for r in res.results])
    return out.reshape(B, 1).astype(np.float32)


if __name__ == "__main__":
    rng = np.random.default_rng(0)
    ins = {
        "dots": rng.integers(0, 6, size=(V, H, B)).astype(np.int32),
        "w_each": rng.standard_normal((FULL, CT), dtype=np.float32) * 0.1,
        "w_not_each": rng.standard_normal((FULL, CT), dtype=np.float32) * 0.1,
        "w_not_each_2nd": rng.standard_normal((FULL, CT), dtype=np.float32) * 0.1,
        "w_empty": rng.standard_normal((FULL, CT), dtype=np.float32) * 0.1,
        "W1": rng.standard_normal((NSQ, NPIX), dtype=np.float32) * 0.2,
        "W2": rng.standard_normal((NSQ, NSQ), dtype=np.float32) * 0.2,
        "b2": rng.standard_normal(NSQ, dtype=np.float32) * 0.1,
        "W3": rng.standard_normal((1, NSQ), dtype=np.float32) * 0.2,
        "b3": rng.standard_normal(1, dtype=np.float32) * 0.1,
    }
    y = kernel(**ins)
    print("kernel out", y.shape, y[:4, 0])


# revision 18
# speedup vs baseline: 1.2338x; 1.2338x over previous
"""Trainium2 Bass kernel for nn_CNN_symmetry (dense_cnn).

Strategy:
  * Pure data parallelism: batch B=32768 sharded across 8 NeuronCores (4096 each).
  * Host-side (numpy): build the 144x144 banded conv matrices from the tiny
    13x4 weights, pre-transpose them into lhsT blocks; rearrange dots into
    [pixel, batch] bf16 planes (main 128 pixels + tail 16 pixels).
  * Device: convs as TensorE matmuls (bf16 in, fp32 PSUM). Convolution pairs
    that share an rhs (cne+cepl on e; cne+csum on m_c) get their 16-row tail
    outputs stacked into one [32,N] psum, saving 2 of 8 matmul passes per pair.
    Elementwise is spread across DVE / ACT / GPSIMD; chunks are stage-skewed
    and double-buffered so engine queues always hold another chunk's work.

Algebraic restructuring (validated vs reference in fp64):
    e=[x==0], m_c=[x==c];  C_sum=C_each+C_ne, C_epl=C_emp+C_ne, T=C_ne@1
    ne_e=C_ne@e ; t0=T-ne_e ; Ew'=C_epl@e-T (=E-t0) ; ie=1-e
    NECn_c = C_ne@m_c - t0        (= -NEC_c)
    s0 = sum_c m_c*(C_sum@m_c + Ew')   [select: masks disjoint]
    all_v = sigmoid(s0);  Ebar = ie*(Ew'+t0)
    2x: g_c=-NECn_c*all_v; s += Ebar + sum_c m_c*(C_ne2@g_c); all_v=tanh(s/2)
    out = lrelu(lrelu(all_v@W1')@W2'+b2)@W3'+b3
"""

import os
import sys
from contextlib import ExitStack

import numpy as np

sys.path.insert(0, "/opt/trn_rl_repo")
os.environ.setdefault("MYCRO_LOCAL_CACHE", "1")

import ml_dtypes  # noqa: E402

import concourse.bass as bass  # noqa: E402
import concourse.bacc as bacc  # noqa: E402
import concourse.tile as tile  # noqa: E402
from concourse import mybir  # noqa: E402

V, H, B = 14, 12, 32768
NK, CT = 5, 4
HALF, FULL = 6, 13
NPIX, NSQ = 144, 100
NCORES = 8
BC = B // NCORES          # 4096 per core
CHUNK = 1024              # batch chunk processed per pipeline pass
NCHUNK = BC // CHUNK
PM, PT = 128, 16          # main/tail pixel split (i-major order p = i*12 + j)

BF16 = mybir.dt.bfloat16
F32 = mybir.dt.float32
I16 = mybir.dt.int16
AF = mybir.ActivationFunctionType
ALU = mybir.AluOpType

# select routing per color (c>=1): A = ACT copy + DVE mult/add, D = DVE pred
S0_ROUTE = ("-", "D", "D", "D", "D")
ACC_ROUTE = ("-", "D", "D", "D", "D")
G_GPS = (False, False, False, False, False)  # GPSIMD elementwise contends with DVE SBUF port: keep off


# ---------------------------------------------------------------- host prep

def _build_K(W):
    Wa = np.abs(np.asarray(W, np.float64))
    K = np.zeros((FULL, FULL))
    K[:, HALF:HALF + CT] = Wa
    K[:, HALF - CT + 1:HALF + 1] = Wa[:, ::-1]
    return K


def _band(K):
    C = np.zeros((NPIX, NPIX))
    for i in range(12):
        for j in range(12):
            for i2 in range(12):
                for j2 in range(12):
                    di, dj = i2 - i + HALF, j2 - j + HALF
                    if 0 <= di < FULL and 0 <= dj < FULL:
                        C[i * 12 + j, i2 * 12 + j2] = K[di, dj]
    return C


def _lhsT(C):
    """C (out,in) -> lhsT = C.T blocks: mm [128,128], tm [16,128], mt [128,16], tt [16,16]."""
    L = C.T.astype(np.float64)
    return {
        "mm": L[:PM, :PM], "tm": L[PM:, :PM],
        "mt": L[:PM, PM:], "tt": L[PM:, PM:],
    }


def build_consts(w_each, w_not_each, w_not_each_2nd, w_empty, W1, W2, b2, W3, b3):
    C_each = _band(_build_K(w_each))
    C_ne = _band(_build_K(w_not_each))
    C_ne2 = _band(_build_K(w_not_each_2nd))
    C_emp = _band(_build_K(w_empty))
    C_sum = C_each + C_ne
    C_epl = C_emp + C_ne
    T = C_ne @ np.ones(NPIX)

    bf = lambda a: np.ascontiguousarray(np.asarray(a), dtype=ml_dtypes.bfloat16)
    f32 = lambda a: np.ascontiguousarray(np.asarray(a), dtype=np.float32)

    Bne, Bsum, Bepl, Bne2 = _lhsT(C_ne), _lhsT(C_sum), _lhsT(C_epl), _lhsT(C_ne2)
    consts = {}
    for nm, Bk in (("cne", Bne), ("csum", Bsum), ("cepl", Bepl), ("cne2", Bne2)):
        consts[f"{nm}_mm"] = bf(Bk["mm"])
        consts[f"{nm}_tm"] = bf(Bk["tm"])
    consts["cne2_mt"] = bf(Bne2["mt"])
    consts["cne2_tt"] = bf(Bne2["tt"])

    def mrg(Ab, Bb, k):
        out = np.zeros((Ab[k].shape[0], 48))
        out[:, :PT] = Ab[k]
        out[:, 32:48] = Bb[k]
        return bf(out)

    consts["necp_mt"] = mrg(Bne, Bepl, "mt")
    consts["necp_tt"] = mrg(Bne, Bepl, "tt")
    consts["necs_mt"] = mrg(Bne, Bsum, "mt")
    consts["necs_tt"] = mrg(Bne, Bsum, "tt")
    consts["t_m"] = f32(T[:PM].reshape(PM, 1))
    consts["t_t"] = f32(T[PM:].reshape(PT, 1))
    consts["tn_m"] = f32(-T[:PM].reshape(PM, 1))
    consts["tn_t"] = f32(-T[PM:].reshape(PT, 1))
    W1T = np.asarray(W1, np.float64).T        # [144, 100]
    consts["w1_m"] = bf(W1T[:PM])
    consts["w1_t"] = bf(W1T[PM:])
    consts["w2"] = bf(np.asarray(W2, np.float64).T)   # [100, 100]
    consts["w3"] = bf(np.asarray(W3, np.float64).T)   # [100, 1]
    consts["b2"] = f32(np.asarray(b2).reshape(NSQ, 1))
    consts["b3"] = f32(np.asarray(b3).reshape(1, 1))
    return consts


CONST_SPECS = (
    [(f"{n}_mm", [PM, PM], BF16) for n in ("cne", "csum", "cepl", "cne2")]
    + [(f"{n}_tm", [PT, PM], BF16) for n in ("cne", "csum", "cepl", "cne2")]
    + [("cne2_mt", [PM, PT], BF16), ("cne2_tt", [PT, PT], BF16),
       ("necp_mt", [PM, 48], BF16), ("necp_tt", [PT, 48], BF16),
       ("necs_mt", [PM, 48], BF16), ("necs_tt", [PT, 48], BF16),
    
       ("t_m", [PM, 1], F32), ("t_t", [PT, 1], F32),
       ("tn_m", [PM, 1], F32), ("tn_t", [PT, 1], F32),
       ("w1_m", [PM, NSQ], BF16), ("w1_t", [PT, NSQ], BF16),
       ("w2", [NSQ, NSQ], BF16), ("w3", [NSQ, 1], BF16),
       ("b2", [NSQ, 1], F32), ("b3", [1, 1], F32)]
)


# ---------------------------------------------------------------- device kernel

def emit_kernel(nc, bc, chunk):
    nchunk = bc // chunk
    xm_d = nc.dram_tensor("xm", [PM, bc], BF16, kind="ExternalInput")
    xt_d = nc.dram_tensor("xt", [PT, bc], BF16, kind="ExternalInput")
    out_d = nc.dram_tensor("out", [1, bc], F32, kind="ExternalOutput")
    const_d = {n: nc.dram_tensor(n, shp, dt, kind="ExternalInput")
               for n, shp, dt in CONST_SPECS}

    with tile.TileContext(nc) as tc, ExitStack() as ctx:
        cpool = ctx.enter_context(tc.tile_pool(name="consts", bufs=1))
        xpool = ctx.enter_context(tc.tile_pool(name="x", bufs=1))
        mpool = ctx.enter_context(tc.tile_pool(name="masks", bufs=2))
        npool = ctx.enter_context(tc.tile_pool(name="necn", bufs=2))
        spool = ctx.enter_context(tc.tile_pool(name="smisc", bufs=2))
        zpool = ctx.enter_context(tc.tile_pool(name="zscr", bufs=2))
        gpool = ctx.enter_context(tc.tile_pool(name="g", bufs=2))
        ppool = ctx.enter_context(tc.tile_pool(name="ps", bufs=2, space="PSUM"))
        tpool = ctx.enter_context(tc.tile_pool(name="pst", bufs=2, space="PSUM"))
        opool = ctx.enter_context(tc.tile_pool(name="outs", bufs=2))

        # ACT table-set warmup (sigmoid/tanh share a set; load it once early).
        warm = cpool.tile([1, 1], F32, tag="warm", name="warm")
        nc.vector.memset(warm[:], 0.0)
        nc.scalar.activation(warm[:], warm[:], AF.Copy)
        nc.scalar.activation(warm[:], warm[:], AF.Sigmoid)
        nc.scalar.activation(warm[:], warm[:], AF.Tanh)

        # first const (feeds PE warmup) + inputs on the sync HWDGE ring, so
        # chunk 0's masks can start ASAP; remaining consts via gpsimd SWDGE.
        C = {}
        n0, shp0, dt0 = CONST_SPECS[0]
        t = cpool.tile(shp0, dt0, tag=n0, name=n0)
        nc.sync.dma_start(t[:], const_d[n0][:])
        C[n0] = t

        xm_f = xpool.tile([PM, bc], BF16, tag="xm", name="xm_t")
        xt_f = xpool.tile([PT, bc], BF16, tag="xt", name="xt_t")
        for ck in range(nchunk):
            sl = slice(ck * chunk, (ck + 1) * chunk)
            nc.sync.dma_start(xm_f[:, sl], xm_d[:, sl])
            nc.sync.dma_start(xt_f[:, sl], xt_d[:, sl])

        for n, shp, dt in CONST_SPECS[1:]:
            t = cpool.tile(shp, dt, tag=n, name=n)
            nc.gpsimd.dma_start(t[:], const_d[n][:])
            C[n] = t

        # PE warmup while input DMAs land (HAM un-throttle needs ~3.4us busy).
        wps = ppool.tile([PM, chunk], F32, tag="psm", name="warm_ps")
        for _ in range(48):
            nc.tensor.matmul(wps[:, :128], C[n0][:], C[n0][:],
                             start=True, stop=True)

        def chunk_stages(ck):
            c0 = ck * chunk
            xm = xm_f[:, c0:c0 + chunk]
            xt = xt_f[:, c0:c0 + chunk]

            # --- S0: masks (bf16 0/1) + ie ---------------------------------
            masks = []  # (main, tail) pairs; masks[0] is e
            for c in range(NK + 1):
                mm_ = mpool.tile([PM, chunk], BF16, tag=f"mkm{c}", name=f"mkm{c}")
                mt_ = mpool.tile([PT, chunk], BF16, tag=f"mkt{c}", name=f"mkt{c}")
                nc.vector.tensor_scalar(mm_[:], xm[:], float(c), None, ALU.is_equal)
                nc.vector.tensor_scalar(mt_[:], xt[:], float(c), None, ALU.is_equal)
                masks.append((mm_, mt_))
            ie = (spool.tile([PM, chunk], BF16, tag="iem", name="iem"),
                  spool.tile([PT, chunk], BF16, tag="iet", name="iet"))
            nc.vector.tensor_scalar(ie[0][:], xm[:], 0.0, None, ALU.not_equal)
            nc.vector.tensor_scalar(ie[1][:], xt[:], 0.0, None, ALU.not_equal)
            yield

            def tiles(tag, pool=spool):
                return (pool.tile([PM, chunk], BF16, tag=tag + "m", name=tag + "m"),
                        pool.tile([PT, chunk], BF16, tag=tag + "t", name=tag + "t"))

            def mm_main(mat, rhs_m, rhs_t):
                """main-output conv part: K=144 split as mm(128)+tm(16)."""
                ps = ppool.tile([PM, chunk], F32, tag="psm", name="psm")
                for nn in range(0, chunk, 512):
                    nc.tensor.matmul(ps[:, nn:nn + 512], C[f"{mat}_mm"][:],
                                     rhs_m[:, nn:nn + 512], start=True, stop=False)
                for nn in range(0, chunk, 512):
                    nc.tensor.matmul(ps[:, nn:nn + 512], C[f"{mat}_tm"][:],
                                     rhs_t[:, nn:nn + 512], start=False, stop=True)
                return ps

            def mm_tail(mat, width, rhs_m, rhs_t):
                """tail-output conv part (possibly merged pair): [width,N]."""
                ps = tpool.tile([48, chunk], F32, tag="pst", name="pst")
                for nn in range(0, chunk, 512):
                    nc.tensor.matmul(ps[:width, nn:nn + 512], C[f"{mat}_mt"][:],
                                     rhs_m[:, nn:nn + 512], start=True, stop=False)
                for nn in range(0, chunk, 512):
                    nc.tensor.matmul(ps[:width, nn:nn + 512], C[f"{mat}_tt"][:],
                                     rhs_t[:, nn:nn + 512], start=False, stop=True)
                return ps

            def zscr(p):
                return (zpool.tile([PM, chunk], BF16, tag="zm", name="zm")
                        if p == 0 else
                        zpool.tile([PT, chunk], BF16, tag="zt", name="zt"))

            # --- S1: e-convs -> t0, Ew', iew, ebar -------------------------
            t0 = tiles("t0")
            ew = tiles("ew")
            ps = mm_main("cne", masks[0][0], masks[0][1])
            nc.scalar.activation(t0[0][:], ps[:], AF.Identity,
                                 bias=C["t_m"][:], scale=-1.0)
            ps = mm_main("cepl", masks[0][0], masks[0][1])
            nc.scalar.activation(ew[0][:], ps[:], AF.Identity, bias=C["tn_m"][:])
            ps = mm_tail("necp", 48, masks[0][0], masks[0][1])
            nc.scalar.activation(t0[1][:], ps[:PT], AF.Identity,
                                 bias=C["t_t"][:], scale=-1.0)
            nc.scalar.activation(ew[1][:], ps[32:48], AF.Identity,
                                 bias=C["tn_t"][:])
            iew = tiles("iew")
            ebar = tiles("ebar")
            for r in range(2):
                nc.vector.tensor_tensor(iew[r][:], ie[r][:], ew[r][:], ALU.mult)
                nc.vector.tensor_tensor(ebar[r][:], ew[r][:], t0[r][:], ALU.add)
                nc.vector.tensor_tensor(ebar[r][:], ebar[r][:], ie[r][:], ALU.mult)
            yield

            # --- S2+S3: per color, merged cne/csum convs:
            #     necn_c = C_ne@m_c - t0 ; s0 = select_c(C_sum@m_c) + iew ----
            necn = [tiles(f"necn{c}", pool=npool) for c in range(NK)]
            s = tiles("s")

            def sel_step(route, dst, mk, src_ap, first, p):
                """dst := select(mk, src) accumulate; masks disjoint."""
                if first:
                    nc.vector.tensor_tensor(dst[:], src_ap, mk[:], ALU.mult)
                elif route == "A":
                    z = zscr(p)
                    nc.scalar.activation(z[:], src_ap, AF.Copy)
                    zz = zscr(p)
                    nc.vector.tensor_tensor(zz[:], z[:], mk[:], ALU.mult)
                    nc.vector.tensor_tensor(dst[:], dst[:], zz[:], ALU.add)
                else:
                    nc.vector.copy_predicated(dst[:], mk[:].bitcast(I16), src_ap)

            def color_pair(c):
                ps = mm_main("cne", masks[c + 1][0], masks[c + 1][1])
                if c >= 3:
                    nc.vector.tensor_tensor(necn[c][0][:], ps[:], t0[0][:],
                                            ALU.subtract)
                else:
                    z = zscr(0)
                    nc.scalar.activation(z[:], ps[:], AF.Copy)
                    nc.vector.tensor_tensor(necn[c][0][:], z[:], t0[0][:],
                                            ALU.subtract)
                ps = mm_main("csum", masks[c + 1][0], masks[c + 1][1])
                sel_step(S0_ROUTE[c], s[0], masks[c + 1][0], ps[:], c == 0, 0)
                ps = mm_tail("necs", 48, masks[c + 1][0], masks[c + 1][1])
                z = zscr(1)
                nc.scalar.activation(z[:], ps[:PT], AF.Copy)
                nc.vector.tensor_tensor(necn[c][1][:], z[:], t0[1][:], ALU.subtract)
                sel_step(S0_ROUTE[c], s[1], masks[c + 1][1], ps[32:48],
                         c == 0, 1)

            for c in range(3):
                color_pair(c)
            yield  # end S2

            for c in range(3, NK):
                color_pair(c)
            allv = tiles("allv")
            for r in range(2):
                nc.vector.tensor_tensor(s[r][:], s[r][:], iew[r][:], ALU.add)
                nc.scalar.activation(allv[r][:], s[r][:], AF.Sigmoid)
            yield  # end S3

            # --- S4/S5: depth iterations -----------------------------------
            for it in range(2):
                acc = tiles("acc")
                for c in range(NK):
                    g = (gpool.tile([PM, chunk], BF16, tag="gm", name="gm"),
                         gpool.tile([PT, chunk], BF16, tag="gt", name="gt"))
                    eng_m = nc.gpsimd if G_GPS[c] else nc.vector
                    eng_m.tensor_tensor(g[0][:], necn[c][0][:], allv[0][:],
                                        ALU.mult)
                    nc.vector.tensor_tensor(g[1][:], necn[c][1][:], allv[1][:],
                                            ALU.mult)
                    ps = mm_main("cne2", g[0], g[1])
                    sel_step(ACC_ROUTE[c], acc[0], masks[c + 1][0], ps[:],
                             c == 0, 0)
                    ps = mm_tail("cne2", PT, g[0], g[1])
                    sel_step(ACC_ROUTE[c], acc[1], masks[c + 1][1], ps[:PT],
                             c == 0, 1)
                for r in range(2):
                    nc.vector.tensor_tensor(s[r][:], s[r][:], ebar[r][:], ALU.add)
                    nc.vector.tensor_tensor(s[r][:], s[r][:], acc[r][:],
                                            ALU.subtract)
                    nc.scalar.activation(allv[r][:], s[r][:], AF.Tanh, scale=0.5)
                yield  # end S4 / S5

            # --- S6: MLP ----------------------------------------------------
            h1 = opool.tile([NSQ, chunk], BF16, tag="h1", name="h1")
            ps = ppool.tile([PM, chunk], F32, tag="psm", name="psm")
            for nn in range(0, chunk, 512):
                sl = slice(nn, nn + 512)
                nc.tensor.matmul(ps[:NSQ, nn:nn + 512], C["w1_m"][:],
                                 allv[0][:, sl], start=True, stop=False)
                nc.tensor.matmul(ps[:NSQ, nn:nn + 512], C["w1_t"][:],
                                 allv[1][:, sl], start=False, stop=True)
            nc.scalar.activation(h1[:], ps[:NSQ], AF.Copy)
            nc.vector.scalar_tensor_tensor(h1[:], ps[:NSQ], 0.2, h1[:],
                                           ALU.mult, ALU.max)

            h2 = opool.tile([NSQ, chunk], BF16, tag="h2", name="h2")
            ps = ppool.tile([PM, chunk], F32, tag="psm", name="psm")
            for nn in range(0, chunk, 512):
                nc.tensor.matmul(ps[:NSQ, nn:nn + 512], C["w2"][:],
                                 h1[:, nn:nn + 512], start=True, stop=True)
            nc.scalar.activation(h2[:], ps[:NSQ], AF.Identity, bias=C["b2"][:])
            nc.vector.scalar_tensor_tensor(h2[:], h2[:], 0.2, h2[:],
                                           ALU.mult, ALU.max)

            yout = opool.tile([1, chunk], F32, tag="yout", name="yout")
            ps = tpool.tile([48, chunk], F32, tag="pst", name="pst")
            for nn in range(0, chunk, 512):
                nc.tensor.matmul(ps[:1, nn:nn + 512], C["w3"][:],
                                 h2[:, nn:nn + 512], start=True, stop=True)
            nc.scalar.activation(yout[:], ps[:1], AF.Identity, bias=C["b3"][:])
            nc.sync.dma_start(out_d[:, c0:c0 + chunk], yout[:])
            yield  # end S6

        # software pipeline: skewed stage interleave across chunks.
        NSTAGE, SKEW = 7, 3
        gens = [chunk_stages(ck) for ck in range(nchunk)]
        sched = sorted((ck * SKEW + st, ck, st)
                       for ck in range(nchunk) for st in range(NSTAGE))
        for _, ck, _s in sched:
            next(gens[ck], None)

    return nc


# ---------------------------------------------------------------- entry point

def _prep_inputs(dots):
    """dots (14,12,B) int32 -> per-core bf16 xm [128, BC], xt [16, BC]."""
    x = np.asarray(dots)[:12].reshape(NPIX, B).astype(ml_dtypes.bfloat16)
    xms, xts = [], []
    for k in range(NCORES):
        sl = x[:, k * BC:(k + 1) * BC]
        xms.append(np.ascontiguousarray(sl[:PM]))
        xts.append(np.ascontiguousarray(sl[PM:]))
    return xms, xts


def kernel(dots, w_each, w_not_each, w_not_each_2nd, w_empty, W1, W2, b2, W3, b3):
    from concourse.bass_utils import run_bass_kernel_spmd

    consts = build_consts(w_each, w_not_each, w_not_each_2nd, w_empty,
                          W1, W2, b2, W3, b3)
    xms, xts = _prep_inputs(dots)

    nc = bacc.Bacc()
    emit_kernel(nc, BC, CHUNK)
    nc.compile()

    in_maps = [dict(consts, xm=xms[k], xt=xts[k]) for k in range(NCORES)]
    res = run_bass_kernel_spmd(nc, in_maps, list(range(NCORES)))
    out = np.concatenate([np.asarray(r["out"]).reshape(BC) for r in res.results])
    return out.reshape(B, 1).astype(np.float32)


if __name__ == "__main__":
    rng = np.random.default_rng(0)
    ins = {
        "dots": rng.integers(0, 6, size=(V, H, B)).astype(np.int32),
        "w_each": rng.standard_normal((FULL, CT), dtype=np.float32) * 0.1,
        "w_not_each": rng.standard_normal((FULL, CT), dtype=np.float32) * 0.1,
        "w_not_each_2nd": rng.standard_normal((FULL, CT), dtype=np.float32) * 0.1,
        "w_empty": rng.standard_normal((FULL, CT), dtype=np.float32) * 0.1,
        "W1": rng.standard_normal((NSQ, NPIX), dtype=np.float32) * 0.2,
        "W2": rng.standard_normal((NSQ, NSQ), dtype=np.float32) * 0.2,
        "b2": rng.standard_normal(NSQ, dtype=np.float32) * 0.1,
        "W3": rng.standard_normal((1, NSQ), dtype=np.float32) * 0.2,
        "b3": rng.standard_normal(1, dtype=np.float32) * 0.1,
    }
    y = kernel(**ins)
    print("kernel out", y.shape, y[:4, 0])
